# revision 19
# baseline (speedup 1.0000x reference)
"""Trainium2 Bass kernel for a Conv2d VQ-VAE model.

Strategy: data-parallel over batch B=128 across 8 NeuronCores (16 samples
per core). All conv weights / codebook replicated. Per-core kernel runs a
per-sample pipeline: encoder convs (BN folded into weights on host) ->
VQ codebook scores + argmin (max8/max_index) -> codebook gather (GPSIMD
ap_gather) -> decoder convs + transposed conv -> state predictor.
Small scalars (losses, perplexity, classifier MLP on pooled codes) are
reduced from tiny per-core partials on the host.
"""

import numpy as np

import concourse.bacc as bacc
import concourse.bass as bass
import concourse.mybir as mybir
import concourse.tile as tile
import concourse.bass_isa as bass_isa
from concourse import library_config

EPS = 1e-5
NCODES = 256
B, T = 128, 1000
NCORES = 8
PC = B // NCORES          # samples per core
NCHUNK = 8                # T partition-chunks of 128 (last=104)
F32 = mybir.dt.float32
I16 = mybir.dt.int16
I32 = mybir.dt.int32
U32 = mybir.dt.uint32
ALU = mybir.AluOpType
ACTF = mybir.ActivationFunctionType

_CACHE = {}


# --------------------------------------------------------------------------
# host-side constant preparation
# --------------------------------------------------------------------------

def _fold_bn(w, b, g, be, m, v):
    s = (g / np.sqrt(v + EPS)).astype(np.float32)
    wf = (w * s[:, None, None, None]).astype(np.float32)
    bf = ((b - m) * s + be).astype(np.float32)
    return wf, bf


def _prep_consts(inp):
    c = {}
    # conv0: (64, 9, 2, 5) -> im2col lhsT [90, 64]; rows = tap*18 + (ci*2+h)
    w0 = inp['enc_w0'].reshape(64, 18, 5)
    c['w0'] = np.ascontiguousarray(w0.transpose(2, 1, 0).reshape(90, 64))
    c['b0'] = inp['enc_b0']
    w1f, b1f = _fold_bn(inp['enc_w1'], inp['enc_b1'], inp['ebn1_g'],
                        inp['ebn1_b'], inp['ebn1_m'], inp['ebn1_v'])
    c['w1'] = np.ascontiguousarray(w1f[:, :, 0, :].transpose(1, 2, 0))      # [64,3,128]
    c['b1'] = b1f
    w2f, b2f = _fold_bn(inp['enc_w2'], inp['enc_b2'], inp['ebn2_g'],
                        inp['ebn2_b'], inp['ebn2_m'], inp['ebn2_v'])
    c['w2'] = np.ascontiguousarray(
        w2f[:, :, 0, :].transpose(1, 2, 0).reshape(128, 3, 2, 128))          # [ci,tap,coh,co]
    c['b2'] = b2f.reshape(2, 128).T.copy()                                   # [128,2]
    wc = inp['enc_wc'][:, :, 0, 0].T                                         # (256,64)
    c['wc'] = np.ascontiguousarray(wc.reshape(2, 128, 64).transpose(1, 0, 2))  # [128,2,64]
    c['bc'] = inp['enc_bc']
    cb = inp['codebook'].astype(np.float32)                                  # (256,64)
    cb2 = np.concatenate([2.0 * cb.T, -(cb * cb).sum(1)[None, :]], axis=0)   # [65,256]
    c['cb2'] = np.ascontiguousarray(cb2.astype(np.float32))
    c['cbg'] = np.ascontiguousarray(cb.T)                                    # [64,256]
    c['wd0'] = np.ascontiguousarray(
        inp['dec_w0'][:, :, 0, 0].T.reshape(64, 2, 128))                     # [64,2,128]
    c['bd0'] = inp['dec_b0'].reshape(2, 128).T.copy()                        # [128,2]
    wd1f, bd1f = _fold_bn(inp['dec_w1'], inp['dec_b1'], inp['dbn1_g'],
                          inp['dbn1_b'], inp['dbn1_m'], inp['dbn1_v'])
    c['wd1'] = np.ascontiguousarray(
        wd1f[:, :, 0, :].transpose(1, 2, 0).reshape(2, 128, 3, 128)
        .transpose(1, 2, 0, 3))                                              # [ci,tap,kh,co]
    c['bd1'] = bd1f
    wd2f, bd2f = _fold_bn(inp['dec_w2'], inp['dec_b2'], inp['dbn2_g'],
                          inp['dbn2_b'], inp['dbn2_m'], inp['dbn2_v'])
    c['wd2'] = np.ascontiguousarray(wd2f[:, :, 0, :].transpose(1, 2, 0))     # [128,3,64]
    c['bd2'] = bd2f
    # ConvTranspose: y[co,h,t] = sum_{kw,ci} dec_wt[ci,co,h,4-kw] d2[ci,t+kw-2]
    # (the transpose-conv H flip cancels against the conv H flip for H=1 input)
    tmp = inp['dec_wt'][:, :, :, ::-1]                                       # [ci,co,h,kw]
    c['wt'] = np.ascontiguousarray(tmp.transpose(0, 3, 1, 2).reshape(64, 5, 18))
    c['bt'] = np.repeat(inp['dec_bt'], 2).astype(np.float32)                 # [18]
    c['wsp1'] = np.ascontiguousarray(inp['sp_w1'][:, :, 0, 0].T)             # [64,32]
    c['bsp1'] = inp['sp_b1']
    c['wsp2'] = np.ascontiguousarray(inp['sp_w2'][:, :, 0, 0].T)             # [32,16]
    c['bsp2'] = inp['sp_b2']
    c['ones'] = np.ones((T,), np.float32)
    return {k: np.ascontiguousarray(v.astype(np.float32) if v.dtype != np.float32 else v)
            for k, v in c.items()}


# --------------------------------------------------------------------------
# device kernel
# --------------------------------------------------------------------------

def _build():
    nc = bacc.Bacc("TRN2", target_bir_lowering=False, debug=False,
                   num_devices=NCORES)

    din = {}
    def dt_in(name, shape, dtype=F32):
        din[name] = nc.dram_tensor(name, list(shape), dtype, kind="ExternalInput")
        return din[name]

    x18 = dt_in('x18', (PC, 18, T))
    for name, shape in [('w0', (90, 64)), ('b0', (64,)),
                        ('w1', (64, 3, 128)), ('b1', (128,)),
                        ('w2', (128, 3, 2, 128)), ('b2', (128, 2)),
                        ('wc', (128, 2, 64)), ('bc', (64,)),
                        ('cb2', (65, 256)), ('cbg', (64, 256)),
                        ('wd0', (64, 2, 128)), ('bd0', (128, 2)),
                        ('wd1', (128, 3, 2, 128)), ('bd1', (128,)),
                        ('wd2', (128, 3, 64)), ('bd2', (64,)),
                        ('wt', (64, 5, 18)), ('bt', (18,)),
                        ('wsp1', (64, 32)), ('bsp1', (32,)),
                        ('wsp2', (32, 16)), ('bsp2', (16,)),
                        ('ones', (T,))]:
        dt_in(name, shape)

    xrec_o = nc.dram_tensor('xrec_o', [PC, 18, T], F32, kind="ExternalOutput")
    states_o = nc.dram_tensor('states_o', [PC, 16, T], F32, kind="ExternalOutput")
    idx_o = nc.dram_tensor('idx_o', [PC, T], I32, kind="ExternalOutput")
    zsq_o = nc.dram_tensor('zsq_o', [64, 1], F32, kind="ExternalOutput")
    smax_o = nc.dram_tensor('smax_o', [128, 1], F32, kind="ExternalOutput")

    with tile.TileContext(nc) as tc:
        _body(tc, din, xrec_o, states_o, idx_o, zsq_o, smax_o)
    nc.compile()
    return nc


def _body(tc, din, xrec_o, states_o, idx_o, zsq_o, smax_o):
    from contextlib import ExitStack
    nc = tc.nc
    es = ExitStack()

    wpool = es.enter_context(tc.tile_pool(name="weights", bufs=1))
    apool = es.enter_context(tc.tile_pool(name="acts", bufs=2))
    ppool = es.enter_context(tc.tile_pool(name="psum", bufs=4, space="PSUM"))
    spool = es.enter_context(tc.tile_pool(name="scorep", bufs=2, space="PSUM"))
    dpool = es.enter_context(tc.tile_pool(name="dram", bufs=2, space="DRAM"))

    nc.gpsimd.load_library(library_config.ap_gather)

    # ---- preload weights / constants ----
    def wload(name, shape, dtype=F32):
        t = wpool.tile(list(shape), dtype, name=f"w_{name}")
        nc.sync.dma_start(out=t[:], in_=din[name].ap())
        return t

    w0 = wload('w0', (90, 64)); b0 = wload('b0', (64, 1))
    w1 = wload('w1', (64, 3, 128)); b1 = wload('b1', (128, 1))
    w2 = wload('w2', (128, 3, 2, 128)); b2 = wload('b2', (128, 2))
    wc = wload('wc', (128, 2, 64)); bc = wload('bc', (64, 1))
    cb2 = wload('cb2', (65, 256))
    cbg = wpool.tile([64, 256, 1], F32, name="w_cbg")
    nc.sync.dma_start(out=cbg[:, :, 0], in_=din['cbg'].ap())
    wd0 = wload('wd0', (64, 2, 128)); bd0 = wload('bd0', (128, 2))
    wd1 = wload('wd1', (128, 3, 2, 128)); bd1 = wload('bd1', (128, 1))
    wd2 = wload('wd2', (128, 3, 64)); bd2 = wload('bd2', (64, 1))
    wt = wload('wt', (64, 5, 18)); bt = wload('bt', (18, 1))
    wsp1 = wload('wsp1', (64, 32)); bsp1 = wload('bsp1', (32, 1))
    wsp2 = wload('wsp2', (32, 16)); bsp2 = wload('bsp2', (16, 1))

    # persistent accumulators
    smax_all = wpool.tile([128, PC * NCHUNK, 8], F32, name="smax_all")
    zsqacc = wpool.tile([64, PC], F32, name="zsqacc")
    nc.vector.memset(smax_all[:], 0.0)
    nc.vector.memset(zsqacc[:], 0.0)

    x18 = din['x18']

    for b in range(PC):
        # ================= encoder =================
        # -- conv0 with DMA im2col: X5[18t+i, j] = x[i, j+t-2]
        X5 = apool.tile([90, T], F32, name="X5")
        nc.vector.memset(X5[0:90, 0:2], 0.0)
        nc.vector.memset(X5[0:90, T - 2:T], 0.0)
        for t in range(5):
            lo = max(0, 2 - t)
            hi = min(T, T + 2 - t)
            nc.sync.dma_start(out=X5[18 * t:18 * t + 18, lo:hi],
                              in_=x18.ap()[b, :, lo + t - 2:hi + t - 2])

        Z0 = apool.tile([64, T + 2], F32, name="Z0")
        nc.vector.memset(Z0[:, 0:1], 0.0)
        nc.vector.memset(Z0[:, T + 1:T + 2], 0.0)
        for cc in range(2):
            z0p = ppool.tile([64, 500], F32, name="pconv")
            nc.tensor.matmul(z0p[:], w0[:], X5[:, 500 * cc:500 * cc + 500],
                             start=True, stop=True)
            nc.scalar.activation(out=Z0[:, 1 + 500 * cc:501 + 500 * cc],
                                 in_=z0p[:], func=ACTF.Identity, bias=b0[:])

        # -- conv1 (64->128, k=3) + bn + relu
        Z1 = apool.tile([128, T + 2], F32, name="Z1")
        nc.vector.memset(Z1[:, 0:1], 0.0)
        nc.vector.memset(Z1[:, T + 1:T + 2], 0.0)
        for cc in range(2):
            z1p = ppool.tile([128, 500], F32, name="pconv")
            for dt in range(3):
                nc.tensor.matmul(z1p[:], w1[:, dt, :],
                                 Z0[:, 500 * cc + dt:500 * cc + dt + 500],
                                 start=(dt == 0), stop=(dt == 2))
            nc.vector.tensor_scalar(Z1[:, 1 + 500 * cc:501 + 500 * cc],
                                    z1p[:], b1[:], 0.0, ALU.add, ALU.max)

        # -- conv2 (128->256, k=3) + bn + relu
        Z2 = [apool.tile([128, T], F32, name=f"Z2_{h}") for h in range(2)]
        for coh in range(2):
            for cc in range(2):
                z2p = ppool.tile([128, 500], F32, name="pconv")
                for dt in range(3):
                    nc.tensor.matmul(z2p[:], w2[:, dt, coh, :],
                                     Z1[:, 500 * cc + dt:500 * cc + dt + 500],
                                     start=(dt == 0), stop=(dt == 2))
                if coh == 0:
                    nc.scalar.activation(out=Z2[coh][:, 500 * cc:500 * cc + 500],
                                         in_=z2p[:], func=ACTF.Relu,
                                         bias=b2[:, coh:coh + 1])
                else:
                    nc.vector.tensor_scalar(Z2[coh][:, 500 * cc:500 * cc + 500],
                                            z2p[:], b2[:, coh:coh + 1], 0.0,
                                            ALU.add, ALU.max)

        # -- convc (256->64, 1x1): z_e  (row 64 = ones for score matmul)
        ZE = apool.tile([65, T], F32, name="ZE")
        for cc in range(2):
            zep = ppool.tile([64, 500], F32, name="pconv")
            for kh in range(2):
                nc.tensor.matmul(zep[:], wc[:, kh, :],
                                 Z2[kh][:, 500 * cc:500 * cc + 500],
                                 start=(kh == 0), stop=(kh == 1))
            nc.scalar.activation(out=ZE[0:64, 500 * cc:500 * cc + 500],
                                 in_=zep[:], func=ACTF.Identity, bias=bc[:])
        nc.sync.dma_start(out=ZE[64:65, :], in_=din['ones'].ap()[None, :])
        # sum of z_e^2 for e_latent (per-sample column)
        scr = apool.tile([64, T], F32, name="scr")
        nc.scalar.activation(out=scr[:], in_=ZE[0:64, :], func=ACTF.Square,
                             accum_out=zsqacc[:, b:b + 1])

        # ================= VQ =================
        NDS = apool.tile([128, NCHUNK, 256], F32, name="NDS")
        idx_all = apool.tile([128, NCHUNK, 8], U32, name="idx_all")
        nc.vector.memset(idx_all[:], 0)
        for ch in range(NCHUNK):
            w = 128 if ch < 7 else T - 128 * 7
            ndp = spool.tile([128, 256], F32, name="psc")
            nc.tensor.matmul(ndp[:w, :], ZE[:, 128 * ch:128 * ch + w], cb2[:],
                             start=True, stop=True)
            nc.scalar.activation(out=NDS[:w, ch, :], in_=ndp[:w, :], func=ACTF.Copy)
            nc.vector.max(smax_all[:w, b * NCHUNK + ch, :], NDS[:w, ch, :])
            nc.vector.max_index(idx_all[:w, ch, :],
                                smax_all[:w, b * NCHUNK + ch, :], NDS[:w, ch, :])

        idx_i32 = apool.tile([128, NCHUNK], I32, name="idx_i32")
        idx_i16 = apool.tile([128, NCHUNK], I16, name="idx_i16")
        nc.vector.tensor_copy(idx_i32[:], idx_all[:, :, 0])
        nc.vector.tensor_copy(idx_i16[:], idx_all[:, :, 0])
        # indices output (n = 128*ch + p)
        nc.sync.dma_start(out=idx_o.ap()[b, 0:896].rearrange("(c p) -> p c", p=128),
                          in_=idx_i32[:, 0:7])
        nc.sync.dma_start(out=idx_o.ap()[b, 896:T],
                          in_=idx_i32[0:104, 7:8])

        # wrapped int16 index layout for ap_gather:
        # scratch2[n] (n-linear) -> scratch[s, 16u+r] = idx[16s+r] -> xbar
        # transpose -> wrapped[p, s] = idx[16s + p%16]
        scratch2 = dpool.tile([1024], I16, name="sc2")
        scratch = dpool.tile([64, 128], I16, name="sc")
        nc.sync.dma_start(out=scratch2[:].rearrange("(c p) -> p c", p=128),
                          in_=idx_i16[:])
        for u in range(8):
            nc.sync.dma_start(
                out=scratch[:, 16 * u:16 * u + 16],
                in_=scratch2[:].rearrange("(s r) -> s r", r=16))
        wrapped = apool.tile([128, 64], I16, name="wrapped")
        nc.sync.dma_start_transpose(wrapped[:], scratch[:])

        ZQ = apool.tile([64, 1024, 1], F32, name="ZQ")
        nc.gpsimd.ap_gather(ZQ[:], cbg[:],
                            wrapped[0:64, :], channels=64, num_elems=256,
                            d=1, num_idxs=1024)
        zq = ZQ[:, :, 0]

        # ================= decoder =================
        # -- dec0 (64->256, 1x1) + bias
        D0 = [apool.tile([128, T + 2], F32, name=f"D0_{h}") for h in range(2)]
        for coh in range(2):
            nc.vector.memset(D0[coh][:, 0:1], 0.0)
            nc.vector.memset(D0[coh][:, T + 1:T + 2], 0.0)
            for cc in range(2):
                d0p = ppool.tile([128, 500], F32, name="pconv")
                nc.tensor.matmul(d0p[:], wd0[:, coh, :],
                                 zq[:, 500 * cc:500 * cc + 500],
                                 start=True, stop=True)
                nc.scalar.activation(
                    out=D0[coh][:, 1 + 500 * cc:501 + 500 * cc],
                    in_=d0p[:], func=ACTF.Identity, bias=bd0[:, coh:coh + 1])

        # -- dec1 (256->128, k=3) + bn + relu
        D1 = apool.tile([128, T + 2], F32, name="D1")
        nc.vector.memset(D1[:, 0:1], 0.0)
        nc.vector.memset(D1[:, T + 1:T + 2], 0.0)
        for cc in range(2):
            d1p = ppool.tile([128, 500], F32, name="pconv")
            first = True
            for kh in range(2):
                for dt in range(3):
                    nc.tensor.matmul(d1p[:], wd1[:, dt, kh, :],
                                     D0[kh][:, 500 * cc + dt:500 * cc + dt + 500],
                                     start=first, stop=(kh == 1 and dt == 2))
                    first = False
            nc.vector.tensor_scalar(D1[:, 1 + 500 * cc:501 + 500 * cc],
                                    d1p[:], bd1[:], 0.0, ALU.add, ALU.max)

        # -- dec2 (128->64, k=3) + bn + relu ; halo 2 for convT
        D2 = apool.tile([64, T + 4], F32, name="D2")
        nc.vector.memset(D2[:, 0:2], 0.0)
        nc.vector.memset(D2[:, T + 2:T + 4], 0.0)
        for cc in range(2):
            d2p = ppool.tile([64, 500], F32, name="pconv")
            for dt in range(3):
                nc.tensor.matmul(d2p[:], wd2[:, dt, :],
                                 D1[:, 500 * cc + dt:500 * cc + dt + 500],
                                 start=(dt == 0), stop=(dt == 2))
            nc.scalar.activation(out=D2[:, 2 + 500 * cc:502 + 500 * cc],
                                 in_=d2p[:], func=ACTF.Relu, bias=bd2[:])

        # -- convT (64->18, k=5) + bias
        XR = apool.tile([18, T], F32, name="XR")
        for cc in range(2):
            xp = ppool.tile([18, 500], F32, name="pconv")
            for kw in range(5):
                nc.tensor.matmul(xp[:], wt[:, kw, :],
                                 D2[:, 500 * cc + kw:500 * cc + kw + 500],
                                 start=(kw == 0), stop=(kw == 4))
            nc.scalar.activation(out=XR[:, 500 * cc:500 * cc + 500],
                                 in_=xp[:], func=ACTF.Identity, bias=bt[:])
        nc.sync.dma_start(out=xrec_o.ap()[b], in_=XR[:])

        # -- state predictor (1x1 convs on zq)
        S1 = apool.tile([32, T], F32, name="S1")
        for cc in range(2):
            s1p = ppool.tile([32, 500], F32, name="pconv")
            nc.tensor.matmul(s1p[:], wsp1[:],
                             zq[:, 500 * cc:500 * cc + 500], start=True, stop=True)
            nc.vector.tensor_scalar(S1[:, 500 * cc:500 * cc + 500],
                                    s1p[:], bsp1[:], 0.0, ALU.add, ALU.max)
        SO = apool.tile([16, T], F32, name="SO")
        for cc in range(2):
            s2p = ppool.tile([16, 500], F32, name="pconv")
            nc.tensor.matmul(s2p[:], wsp2[:],
                             S1[:, 500 * cc:500 * cc + 500], start=True, stop=True)
            nc.vector.tensor_scalar_add(SO[:, 500 * cc:500 * cc + 500],
                                        s2p[:], bsp2[:])
        nc.sync.dma_start(out=states_o.ap()[b], in_=SO[:])

    # ---- final partial reductions ----
    zsq_col = wpool.tile([64, 1], F32, name="zsq_col")
    nc.vector.reduce_sum(zsq_col[:], zsqacc[:], axis=mybir.AxisListType.X)
    nc.sync.dma_start(out=zsq_o.ap(), in_=zsq_col[:])
    smax_col = wpool.tile([128, 1], F32, name="smax_col")
    nc.vector.reduce_sum(smax_col[:], smax_all[:, :, 0],
                         axis=mybir.AxisListType.X)
    nc.sync.dma_start(out=smax_o.ap(), in_=smax_col[:])
    es.close()


# --------------------------------------------------------------------------
# runner (jit cached so repeated calls don't recompile)
# --------------------------------------------------------------------------

def _get_exec():
    if 'exec' in _CACHE:
        return _CACHE['exec']
    import jax
    import jax.numpy as jnp
    from jax.sharding import Mesh, PartitionSpec
    from jax.experimental.shard_map import shard_map
    from concourse import bass2jax

    nc = _build()
    bass2jax.install_neuronx_cc_hook()

    partition_name = nc.partition_id_tensor.name if nc.partition_id_tensor else None
    in_names, out_names, out_avals, zero_outs = [], [], [], []
    for alloc in nc.m.functions[0].allocations:
        if not isinstance(alloc, mybir.MemoryLocationSet):
            continue
        name = alloc.memorylocations[0].name
        if alloc.kind == "ExternalInput":
            if name != partition_name:
                in_names.append(name)
        elif alloc.kind == "ExternalOutput":
            np_dt = mybir.dt.np(alloc.dtype)
            out_names.append(name)
            out_avals.append(jax.core.ShapedArray(tuple(alloc.tensor_shape), np_dt))
            zero_outs.append(np.zeros(tuple(alloc.tensor_shape), np_dt))

    n_params = len(in_names)
    n_outs = len(out_names)
    all_in_names = list(in_names) + list(out_names)
    if partition_name is not None:
        all_in_names.append(partition_name)

    def _body_fn(*args):
        operands = list(args)
        if partition_name is not None:
            operands.append(bass2jax.partition_id_tensor())
        outs = bass2jax._bass_exec_p.bind(
            *operands,
            out_avals=tuple(out_avals),
            in_names=tuple(all_in_names),
            out_names=tuple(out_names),
            lowering_input_output_aliases=(),
            sim_require_finite=True,
            sim_require_nnan=True,
            nc=nc,
        )
        return tuple(outs)

    devices = jax.devices()[:NCORES]
    mesh = Mesh(np.asarray(devices), ("core",))
    donate = tuple(range(n_params, n_params + n_outs))
    sharded = jax.jit(
        shard_map(_body_fn, mesh=mesh,
                  in_specs=(PartitionSpec("core"),) * (n_params + n_outs),
                  out_specs=(PartitionSpec("core"),) * n_outs,
                  check_rep=False),
        donate_argnums=donate, keep_unused=True)

    def run(in_maps):
        concat_in = [np.concatenate([np.asarray(m[n]) for m in in_maps], axis=0)
                     for n in in_names]
        concat_zero = [np.zeros((NCORES * z.shape[0], *z.shape[1:]), z.dtype)
                       for z in zero_outs]
        out_arrs = sharded(*concat_in, *concat_zero)
        return [
            {n: np.asarray(out_arrs[i]).reshape(NCORES, *out_avals[i].shape)[c]
             for i, n in enumerate(out_names)}
            for c in range(NCORES)
        ]

    _CACHE['exec'] = run
    return run


# --------------------------------------------------------------------------
# public entry point
# --------------------------------------------------------------------------

def kernel(**inputs):
    inputs = {k: np.asarray(v) for k, v in inputs.items()}
    consts = _prep_consts(inputs)
    x = inputs['x'].astype(np.float32).reshape(B, 18, T)

    run = _get_exec()
    in_maps = []
    for c in range(NCORES):
        m = dict(consts)
        m['x18'] = np.ascontiguousarray(x[c * PC:(c + 1) * PC])
        in_maps.append(m)
    results = run(in_maps)

    # ---- host-side unshard + tiny reductions ----
    x_recon = np.concatenate([r['xrec_o'] for r in results], axis=0)
    x_recon = x_recon.reshape(B, 9, 2, T)
    states = np.concatenate([r['states_o'] for r in results], axis=0)
    states = states.reshape(B, 16, 1, T)
    idx = np.concatenate([r['idx_o'] for r in results], axis=0).astype(np.int32)
    indices = idx.reshape(B, 1, T)

    zsq_sum = np.sum([r['zsq_o'].astype(np.float64).sum() for r in results])
    smax_sum = np.sum([r['smax_o'].astype(np.float64).sum() for r in results])
    N = B * T
    e_latent = np.float32((zsq_sum - smax_sum) / (N * 64))
    vq_loss = np.float32(0.4 * e_latent)

    counts = np.bincount(idx.reshape(-1), minlength=NCODES).astype(np.float32)
    avg = counts / np.float32(N)
    perplexity = np.float32(np.exp(-np.sum(avg * np.log(avg + 1e-10))))
    usage = np.float32(np.mean((avg > 0).astype(np.float32)))

    cb = inputs['codebook'].astype(np.float32)
    counts_b = np.zeros((B, NCODES), np.float32)
    for bb in range(B):
        counts_b[bb] = np.bincount(idx[bb], minlength=NCODES)
    pooled = counts_b @ cb / np.float32(T)
    h = np.maximum(pooled @ inputs['cls_w1'].T + inputs['cls_b1'], 0.0)
    activity_logits = (h @ inputs['cls_w2'].T + inputs['cls_b2']).astype(np.float32)

    return (x_recon, activity_logits, vq_loss, np.float32(e_latent),
            perplexity, usage, indices, states)
